# revision 8
# baseline (speedup 1.0000x reference)
"""DispersionLoss (InfoNCE_l2 variant) on 8 Trainium2 NeuronCores.

Computes  log( E_{i!=j}[ exp(-||z_i - z_j||^2 / tau) ] )  for z [8192, 512] fp32.

Strategy
--------
Let y = z * sqrt(2/tau), sqy_i = ||y_i||^2. Then
    exp(-||z_i-z_j||^2/tau) = exp(y_i.y_j - sqy_i/2 - sqy_j/2)
(the relu clamp in the reference only matters on the diagonal, which we mask).

The 8192x8192 pair matrix is tiled into a 16x16 grid of 512x512 blocks.
Using symmetry, each unordered off-diagonal block pair is computed once:
core c owns block-rows {c, c+8} and computes blocks
    (c,   c+d) for d=0..8   and   (c+8, c+8+d mod 16) for d=0..7
which partitions { diag blocks } + { unordered pairs } exactly across 8 cores
(17 block-tiles per core). Off-diag block sums get host weight 2, diag blocks
weight 1 (with their true diagonal masked out on-device via an identity-matmul
that adds -50 to the pre-exp argument).

SPMD trick: every core receives y^T with its columns *rotated* by 512*c, so
the schedule (which local column block pairs with which local lhs block) is
identical on every core; only the data differs. The lhsT tiles are slices of
the same rotated y^T already resident in SBUF (local blocks L0 and L8).

Per psum tile [128,512]: 4 bf16 matmuls (K=128 each) accumulate y_i.y_j,
one K=2 matmul adds -sqy_j/2 (bf16 Dekker hi+lo rows for fp32-level accuracy),
then ScalarE computes exp(arg + bias) with the per-partition bias -sqy_i/2 and
a fused row-sum (accum_out) into a [128, 68] stats buffer. Host applies block
weights, the row factor is already in the bias, and takes log(sum / (N*(N-1))).
"""

import math

import numpy as np
import ml_dtypes

TAU = 100.0
N = 8192
DIM = 512
NCORES = 8
BLK = 512          # block size (rows/cols of a block-tile)
NBLK = 16          # number of 512-blocks along each axis
P = 128
KCH = 4            # contraction chunks of 128
NQ = 17            # block-tiles per core
NT = 4 * NQ        # psum tiles per core
DIAG_QUADS = (0, 9)
DIAG_NEG = -50.0   # added to pre-exp argument on the true diagonal

_cache = {}


def _build_nc():
    import concourse.bass as bass
    import concourse.bacc as bacc
    import concourse.mybir as mybir
    from concourse.tile import TileContext

    bf16 = mybir.dt.bfloat16
    f32 = mybir.dt.float32
    Exp = mybir.ActivationFunctionType.Exp

    nc = bacc.Bacc(trn_type="TRN2")

    y = nc.dram_tensor("y", [DIM, N], bf16, kind="ExternalInput")
    nsq = nc.dram_tensor("nsq", [2, N], bf16, kind="ExternalInput")
    bias = nc.dram_tensor("bias", [P, 8], f32, kind="ExternalInput")
    ones2 = nc.dram_tensor("ones2", [2, P], bf16, kind="ExternalInput")
    ident = nc.dram_tensor("ident", [P, P], bf16, kind="ExternalInput")
    dpat = nc.dram_tensor("dpat", [P, 4 * BLK], bf16, kind="ExternalInput")
    stats = nc.dram_tensor("stats", [P, NT], f32, kind="ExternalOutput")

    # block-tile schedule: (lhs block index {0: local L0, 1: local L8}, local
    # col block, is_diag). Identical on every core thanks to the rotation.
    quads = (
        [(0, 0, True)]
        + [(0, L, False) for L in range(1, 9)]
        + [(1, 8, True)]
        + [(1, L, False) for L in range(9, 16)]
    )

    with TileContext(nc) as tc:
        with (
            tc.tile_pool(name="persist", bufs=1) as pp,
            tc.tile_pool(name="psum", bufs=8, space="PSUM") as psp,
        ):
            rhs = [
                [pp.tile([P, BLK], bf16, tag=f"rhs_{k}_{L}", name=f"rhs_{k}_{L}") for L in range(NBLK)]
                for k in range(KCH)
            ]
            nsq_t = pp.tile([2, N], bf16, tag="nsq", name="nsq_t")
            bias_t = pp.tile([P, 8], f32, tag="bias", name="bias_t")
            ones2_t = pp.tile([2, P], bf16, tag="ones2", name="ones2_t")
            ident_t = pp.tile([P, P], bf16, tag="ident", name="ident_t")
            dpat_t = pp.tile([P, 4 * BLK], bf16, tag="dpat", name="dpat_t")
            stats_t = pp.tile([P, NT], f32, tag="stats", name="stats_t")
            e_t = pp.tile([P, BLK], bf16, tag="e", name="e_t")
            warm_t = pp.tile([P, 8], f32, tag="warm", name="warm_t")

            nc.sync.dma_start(nsq_t[:], nsq[:, :])
            nc.sync.dma_start(bias_t[:], bias[:, :])
            nc.sync.dma_start(ones2_t[:], ones2[:, :])
            nc.sync.dma_start(ident_t[:], ident[:, :])
            nc.sync.dma_start(dpat_t[:], dpat[:, :])
            # ScalarE observes the bias DMA here so later activations only
            # need the PE wait (activation structs have few wait slots).
            nc.scalar.copy(warm_t[:], bias_t[:])
            for L in range(NBLK):
                for k in range(KCH):
                    nc.sync.dma_start(
                        rhs[k][L][:],
                        y[k * P : (k + 1) * P, L * BLK : (L + 1) * BLK],
                    )

            for q, (lhs_idx, L, is_diag) in enumerate(quads):
                for rt_ in range(4):
                    rt = 4 * lhs_idx + rt_
                    t = 4 * q + rt_
                    ps = psp.tile([P, BLK], f32, tag="ps", name=f"ps_{q}_{rt_}")
                    for k in range(KCH):
                        lhsrc = rhs[k][0] if lhs_idx == 0 else rhs[k][8]
                        nc.tensor.matmul(
                            ps[:],
                            lhsrc[:, rt_ * P : (rt_ + 1) * P],
                            rhs[k][L][:],
                            start=(k == 0),
                            stop=False,
                        )
                    nc.tensor.matmul(
                        ps[:],
                        ones2_t[:],
                        nsq_t[:, L * BLK : (L + 1) * BLK],
                        start=False,
                        stop=not is_diag,
                    )
                    if is_diag:
                        nc.tensor.matmul(
                            ps[:],
                            ident_t[:],
                            dpat_t[:, rt_ * BLK : (rt_ + 1) * BLK],
                            start=False,
                            stop=True,
                        )
                    nc.scalar.activation(
                        e_t[:],
                        ps[:],
                        Exp,
                        bias=bias_t[:, rt : rt + 1],
                        scale=1.0,
                        accum_out=stats_t[:, t : t + 1],
                    )

            nc.sync.dma_start(stats[:, :], stats_t[:])

    nc.compile()
    return nc


def _host_inputs(z: np.ndarray):
    """Build the per-core input maps from the full z [8192, 512] fp32."""
    bf16 = ml_dtypes.bfloat16
    z64 = z.astype(np.float64)
    s = math.sqrt(2.0 / TAU)
    yT64 = (z64 * s).T  # [512, 8192]
    sqy64 = (2.0 / TAU) * np.sum(z64 * z64, axis=1)  # [8192]
    v64 = -0.5 * sqy64  # -sqy_j / 2

    ones2 = np.ones((2, P), dtype=bf16)
    ident = np.eye(P, dtype=np.float32).astype(bf16)
    dpat = np.zeros((P, 4 * BLK), dtype=np.float32)
    for rt_ in range(4):
        for p in range(P):
            dpat[p, rt_ * BLK + rt_ * P + p] = DIAG_NEG
    dpat = dpat.astype(bf16)

    in_maps = []
    for c in range(NCORES):
        yr = np.roll(yT64, -BLK * c, axis=1)
        vr = np.roll(v64, -BLK * c)
        hi = vr.astype(np.float32).astype(bf16)
        lo = (vr - hi.astype(np.float64)).astype(np.float32).astype(bf16)
        nsq = np.stack([hi, lo], axis=0)  # [2, N]

        bias = np.empty((P, 8), dtype=np.float32)
        for rt in range(8):
            base = BLK * (c + 8 * (rt // 4)) + (rt % 4) * P
            bias[:, rt] = v64[base : base + P].astype(np.float32)

        in_maps.append(
            {
                "y": np.ascontiguousarray(yr.astype(np.float32).astype(bf16)),
                "nsq": nsq,
                "bias": bias,
                "ones2": ones2,
                "ident": ident,
                "dpat": dpat,
            }
        )
    return in_maps


def _reduce(results) -> np.ndarray:
    total = 0.0
    for out_map in results:
        st = out_map["stats"].astype(np.float64)  # [P, NT]
        persum = st.sum(axis=0)  # [NT]
        for q in range(NQ):
            w = 1.0 if q in DIAG_QUADS else 2.0
            total += w * persum[4 * q : 4 * q + 4].sum()
    mean = total / (float(N) * float(N - 1))
    return np.array(math.log(mean), dtype=np.float32)


def run(z: np.ndarray, trace: bool = False, tmpdir=None):
    from concourse.bass_utils import run_bass_kernel_spmd

    if "nc" not in _cache:
        _cache["nc"] = _build_nc()
    nc = _cache["nc"]
    in_maps = _host_inputs(np.asarray(z, dtype=np.float32))
    res = run_bass_kernel_spmd(
        nc, in_maps, core_ids=list(range(NCORES)), trace=trace, tmpdir=tmpdir
    )
    return _reduce(res.results), res


def kernel(z: np.ndarray) -> np.ndarray:
    out, _ = run(z, trace=False)
    return out


# revision 9
# speedup vs baseline: 1.2105x; 1.2105x over previous
"""DispersionLoss (InfoNCE_l2 variant) on 8 Trainium2 NeuronCores.

Computes  log( E_{i!=j}[ exp(-||z_i - z_j||^2 / tau) ] )  for z [8192, 512] fp32.

Strategy
--------
Let y = z * sqrt(2/tau), sqy_i = ||y_i||^2. Then
    exp(-||z_i-z_j||^2/tau) = exp(y_i.y_j - sqy_i/2 - sqy_j/2)
(the relu clamp in the reference only matters on the diagonal, which we mask).

The 8192x8192 pair matrix is tiled into a 16x16 grid of 512x512 blocks.
Using symmetry, each unordered off-diagonal block pair is computed once:
core c owns block-rows {c, c+8} and computes blocks
    (c,   c+d) for d=0..8   and   (c+8, c+8+d mod 16) for d=0..7
which partitions { diag blocks } + { unordered pairs } exactly across 8 cores
(17 block-tiles per core). Off-diag block sums get host weight 2, diag blocks
weight 1 (with their true diagonal masked out on-device via an identity-matmul
that adds -50 to the pre-exp argument).

SPMD trick: every core receives y^T with its columns *rotated* by 512*c, so
the schedule (which local column block pairs with which local lhs block) is
identical on every core; only the data differs. The lhsT tiles are slices of
the same rotated y^T already resident in SBUF (local blocks L0 and L8).

Per psum tile [128,512]: 4 bf16 matmuls (K=128 each) accumulate y_i.y_j, one
zero-padded K=128 matmul adds -sqy_j/2 (bf16 Dekker hi+lo rows; K=128 keeps
the weight load FWL-eligible so it overlaps the previous matmul), then ScalarE
computes exp(arg + bias) in place on PSUM with per-partition bias -sqy_i/2,
and VectorE reduces each tile to a column of a [128, 68] stats buffer.
Host applies block weights and takes log(sum / (N*(N-1))).

The y input is laid out [16, 128, 4*512] (column-block major, contraction
chunk along the free dim) so each 512KB column block is one dense DMA.
A few warm-up matmuls on memset data run while DMAs stream so the PE's HAM
clock gate is already open (2.4GHz) when the real matmuls start.
"""

import math

import numpy as np
import ml_dtypes

TAU = 100.0
N = 8192
DIM = 512
NCORES = 8
BLK = 512          # block size (rows/cols of a block-tile)
NBLK = 16          # number of 512-blocks along each axis
P = 128
KCH = 4            # contraction chunks of 128
NQ = 17            # block-tiles per core
NT = 4 * NQ        # psum tiles per core
DIAG_QUADS = (0, 9)
DIAG_NEG = -50.0   # added to pre-exp argument on the true diagonal
N_WARMUP_MM = 10

_cache = {}


def _build_nc():
    import concourse.bacc as bacc
    import concourse.mybir as mybir
    from concourse.tile import TileContext

    bf16 = mybir.dt.bfloat16
    f32 = mybir.dt.float32
    Exp = mybir.ActivationFunctionType.Exp
    X = mybir.AxisListType.X

    nc = bacc.Bacc(trn_type="TRN2")

    y = nc.dram_tensor("y", [NBLK, P, KCH * BLK], bf16, kind="ExternalInput")
    nsq = nc.dram_tensor("nsq", [P, N], bf16, kind="ExternalInput")
    bias = nc.dram_tensor("bias", [P, 8], f32, kind="ExternalInput")
    ones = nc.dram_tensor("ones", [P, P], bf16, kind="ExternalInput")
    ident = nc.dram_tensor("ident", [P, P], bf16, kind="ExternalInput")
    dpat = nc.dram_tensor("dpat", [P, 4 * BLK], bf16, kind="ExternalInput")
    stats = nc.dram_tensor("stats", [P, NT], f32, kind="ExternalOutput")

    # block-tile schedule: (lhs block index {0: local L0, 1: local L8}, local
    # col block, is_diag). Identical on every core thanks to the rotation.
    quads = (
        [(0, 0, True)]
        + [(0, L, False) for L in range(1, 9)]
        + [(1, 8, True)]
        + [(1, L, False) for L in range(9, 16)]
    )

    with TileContext(nc) as tc:
        with (
            tc.tile_pool(name="persist", bufs=1) as pp,
            tc.tile_pool(name="psum", bufs=8, space="PSUM") as psp,
        ):
            rhs = [
                pp.tile([P, KCH * BLK], bf16, tag=f"rhs_{L}", name=f"rhs_{L}")
                for L in range(NBLK)
            ]
            nsq_t = pp.tile([P, N], bf16, tag="nsq", name="nsq_t")
            bias_t = pp.tile([P, 8], f32, tag="bias", name="bias_t")
            ones_t = pp.tile([P, P], bf16, tag="ones", name="ones_t")
            ident_t = pp.tile([P, P], bf16, tag="ident", name="ident_t")
            dpat_t = pp.tile([P, 4 * BLK], bf16, tag="dpat", name="dpat_t")
            stats_t = pp.tile([P, NT], f32, tag="stats", name="stats_t")
            warm_t = pp.tile([P, 8], f32, tag="warm", name="warm_t")
            wsrc_t = pp.tile([P, BLK], bf16, tag="wsrc", name="wsrc_t")

            # PE warm-up on memset data (no DMA dependency): opens the HAM
            # clock gate while the first column blocks stream in.
            nc.gpsimd.memset(wsrc_t[:], 0.0)
            wps = psp.tile([P, BLK], f32, tag="ps", name="warm_ps")
            for i in range(N_WARMUP_MM):
                nc.tensor.matmul(
                    wps[:], wsrc_t[:, :P], wsrc_t[:], start=True, stop=True
                )

            nc.sync.dma_start(rhs[0][:], y[0])
            nc.sync.dma_start(nsq_t[:], nsq[:, :])
            nc.sync.dma_start(bias_t[:], bias[:, :])
            nc.sync.dma_start(ones_t[:], ones[:, :])
            nc.sync.dma_start(ident_t[:], ident[:, :])
            nc.sync.dma_start(dpat_t[:], dpat[:, :])
            # ScalarE observes the bias DMA here so the activations only
            # need the PE wait.
            nc.scalar.copy(warm_t[:], bias_t[:])
            for L in range(1, NBLK):
                nc.sync.dma_start(rhs[L][:], y[L])

            for q, (lhs_idx, L, is_diag) in enumerate(quads):
                lhsrc = rhs[0] if lhs_idx == 0 else rhs[8]
                for rt_ in range(4):
                    rt = 4 * lhs_idx + rt_
                    t = 4 * q + rt_
                    ps = psp.tile([P, BLK], f32, tag="ps", name=f"ps_{q}_{rt_}")
                    for k in range(KCH):
                        nc.tensor.matmul(
                            ps[:],
                            lhsrc[:, k * BLK + rt_ * P : k * BLK + (rt_ + 1) * P],
                            rhs[L][:, k * BLK : (k + 1) * BLK],
                            start=(k == 0),
                            stop=False,
                        )
                    nc.tensor.matmul(
                        ps[:],
                        ones_t[:],
                        nsq_t[:, L * BLK : (L + 1) * BLK],
                        start=False,
                        stop=not is_diag,
                    )
                    if is_diag:
                        nc.tensor.matmul(
                            ps[:],
                            ident_t[:],
                            dpat_t[:, rt_ * BLK : (rt_ + 1) * BLK],
                            start=False,
                            stop=True,
                        )
                    nc.scalar.activation(
                        ps[:],
                        ps[:],
                        Exp,
                        bias=bias_t[:, rt : rt + 1],
                        scale=1.0,
                    )
                    nc.vector.reduce_sum(
                        stats_t[:, t : t + 1], ps[:], axis=X
                    )

            nc.sync.dma_start(stats[:, :], stats_t[:])

    nc.compile()
    return nc


def _host_inputs(z: np.ndarray):
    """Build the per-core input maps from the full z [8192, 512] fp32."""
    bf16 = ml_dtypes.bfloat16
    z64 = z.astype(np.float64)
    s = math.sqrt(2.0 / TAU)
    yT64 = (z64 * s).T  # [512, 8192]
    sqy64 = (2.0 / TAU) * np.sum(z64 * z64, axis=1)  # [8192]
    v64 = -0.5 * sqy64  # -sqy_j / 2

    ones = np.ones((P, P), dtype=bf16)
    ident = np.eye(P, dtype=np.float32).astype(bf16)
    dpat = np.zeros((P, 4 * BLK), dtype=np.float32)
    for rt_ in range(4):
        for p in range(P):
            dpat[p, rt_ * BLK + rt_ * P + p] = DIAG_NEG
    dpat = dpat.astype(bf16)

    in_maps = []
    for c in range(NCORES):
        yr = np.roll(yT64, -BLK * c, axis=1).astype(np.float32).astype(bf16)
        # [512, 8192] -> [L=16, p=128, k=4, c=512] -> [16, 128, 2048]
        yl = np.ascontiguousarray(
            yr.reshape(KCH, P, NBLK, BLK).transpose(2, 1, 0, 3).reshape(
                NBLK, P, KCH * BLK
            )
        )

        vr = np.roll(v64, -BLK * c)
        hi = vr.astype(np.float32).astype(bf16)
        lo = (vr - hi.astype(np.float64)).astype(np.float32).astype(bf16)
        nsq = np.zeros((P, N), dtype=bf16)
        nsq[0] = hi
        nsq[1] = lo

        bias = np.empty((P, 8), dtype=np.float32)
        for rt in range(8):
            base = BLK * (c + 8 * (rt // 4)) + (rt % 4) * P
            bias[:, rt] = v64[base : base + P].astype(np.float32)

        in_maps.append(
            {
                "y": yl,
                "nsq": nsq,
                "bias": bias,
                "ones": ones,
                "ident": ident,
                "dpat": dpat,
            }
        )
    return in_maps


def _reduce(results) -> np.ndarray:
    total = 0.0
    for out_map in results:
        st = out_map["stats"].astype(np.float64)  # [P, NT]
        persum = st.sum(axis=0)  # [NT]
        for q in range(NQ):
            w = 1.0 if q in DIAG_QUADS else 2.0
            total += w * persum[4 * q : 4 * q + 4].sum()
    mean = total / (float(N) * float(N - 1))
    return np.array(math.log(mean), dtype=np.float32)


def run(z: np.ndarray, trace: bool = False, tmpdir=None):
    from concourse.bass_utils import run_bass_kernel_spmd

    if "nc" not in _cache:
        _cache["nc"] = _build_nc()
    nc = _cache["nc"]
    in_maps = _host_inputs(np.asarray(z, dtype=np.float32))
    res = run_bass_kernel_spmd(
        nc, in_maps, core_ids=list(range(NCORES)), trace=trace, tmpdir=tmpdir
    )
    return _reduce(res.results), res


def kernel(z: np.ndarray) -> np.ndarray:
    out, _ = run(z, trace=False)
    return out


# revision 12
# speedup vs baseline: 1.2658x; 1.0456x over previous
"""DispersionLoss (InfoNCE_l2 variant) on 8 Trainium2 NeuronCores.

Computes  log( E_{i!=j}[ exp(-||z_i - z_j||^2 / tau) ] )  for z [8192, 512] fp32.

Strategy
--------
Let y = z * sqrt(2/tau), sqy_i = ||y_i||^2. Then
    exp(-||z_i-z_j||^2/tau) = exp(y_i.y_j) * exp(-sqy_i/2) * exp(-sqy_j/2)
(the relu clamp in the reference only matters on the diagonal, which we mask).

The 8192x8192 pair matrix is tiled into a 16x16 grid of 512x512 blocks.
Using symmetry, each unordered off-diagonal block pair is computed once:
core c owns block-rows {c, c+8} and computes blocks
    (c,   c+d) for d=0..8   and   (c+8, c+8+d mod 16) for d=0..7
which partitions { diag blocks } + { unordered pairs } exactly across 8 cores
(17 block-tiles per core). Off-diag block sums get host weight 2, diag blocks
weight 1 (their true diagonal is masked on-device via an identity-matmul that
adds -50 to the pre-exp argument).

SPMD trick: every core receives y^T with its columns *rotated* by 512*c, so
the schedule (which local column block pairs with which local lhs block) is
identical on every core; only the data differs. The lhsT tiles are slices of
the same rotated y^T already resident in SBUF (local blocks L0 and L8).

Engine split per 512x512 block-tile (a "quad" of 4 psum banks):
  - TensorE: 16 bf16 matmuls (K=128) accumulate G = y_i.y_j into one
    [128, 2048] psum tile (+1 identity-matmul per bank on diag tiles).
  - ScalarE: one Exp activation over the whole [128, 2048] psum tile
    into a bf16 SBUF tile (pure exp, no bias).
  - VectorE: per bank, one fused tensor_tensor_reduce:
    (E * A_colblock) row-summed into a column of a [128, 68] stats buffer,
    where A_j = exp(-sqy_j/2) is a precomputed input.
  - Host: applies the row factor exp(-sqy_i/2), the block weights, and the
    final log(sum / (N*(N-1))).

The y input is laid out [16, 128, 4*512] (column-block major, contraction
chunk along the free dim) so each 512KB column block is one dense DMA.
A few warm-up matmuls on memset data run while the DMAs stream so the PE's
HAM clock gate is already open (2.4GHz) when the real matmuls start.
"""

import math

import numpy as np
import ml_dtypes

TAU = 100.0
N = 8192
DIM = 512
NCORES = 8
BLK = 512          # block size (rows/cols of a block-tile)
NBLK = 16          # number of 512-blocks along each axis
P = 128
KCH = 4            # contraction chunks of 128
NQ = 17            # block-tiles per core
NT = 4 * NQ        # psum tiles per core
DIAG_QUADS = (0, 9)
DIAG_NEG = -50.0   # added to pre-exp argument on the true diagonal
N_WARMUP_MM = 12

_cache = {}


def _build_nc():
    import concourse.bacc as bacc
    import concourse.mybir as mybir
    from concourse.tile import TileContext

    bf16 = mybir.dt.bfloat16
    f32 = mybir.dt.float32
    Exp = mybir.ActivationFunctionType.Exp
    mult = mybir.AluOpType.mult
    X = mybir.AxisListType.X

    nc = bacc.Bacc(trn_type="TRN2")

    y = nc.dram_tensor("y", [NBLK, P, KCH * BLK], bf16, kind="ExternalInput")
    acol = nc.dram_tensor("acol", [P, N], bf16, kind="ExternalInput")
    ident = nc.dram_tensor("ident", [P, P], bf16, kind="ExternalInput")
    dpat = nc.dram_tensor("dpat", [P, 4 * BLK], bf16, kind="ExternalInput")
    stats = nc.dram_tensor("stats", [P, NT], f32, kind="ExternalOutput")

    # block-tile schedule: (lhs block index {0: local L0, 1: local L8}, local
    # col block, is_diag). Identical on every core thanks to the rotation.
    quads = (
        [(0, 0, True)]
        + [(0, L, False) for L in range(1, 9)]
        + [(1, 8, True)]
        + [(1, L, False) for L in range(9, 16)]
    )

    with TileContext(nc) as tc:
        with (
            tc.tile_pool(name="persist", bufs=1) as pp,
            tc.tile_pool(name="equad", bufs=3) as ep,
            tc.tile_pool(name="psum", bufs=2, space="PSUM") as psp,
        ):
            rhs = [
                pp.tile([P, KCH * BLK], bf16, tag=f"rhs_{L}", name=f"rhs_{L}")
                for L in range(NBLK)
            ]
            acol_t = pp.tile([P, N], bf16, tag="acol", name="acol_t")
            ident_t = pp.tile([P, P], bf16, tag="ident", name="ident_t")
            dpat_t = pp.tile([P, 4 * BLK], bf16, tag="dpat", name="dpat_t")
            stats_t = pp.tile([P, NT], f32, tag="stats", name="stats_t")
            wsrc_t = pp.tile([P, BLK], bf16, tag="wsrc", name="wsrc_t")

            # PE warm-up on memset data (no DMA dependency): opens the HAM
            # clock gate while the first column blocks stream in.
            nc.gpsimd.memset(wsrc_t[:], 0.0)
            wps = psp.tile([P, 4 * BLK], f32, tag="ps", name="warm_ps")
            for i in range(N_WARMUP_MM):
                nc.tensor.matmul(
                    wps[:, :BLK], wsrc_t[:, :P], wsrc_t[:], start=True, stop=True
                )

            nc.sync.dma_start(rhs[0][:], y[0])
            nc.sync.dma_start(ident_t[:], ident[:, :])
            nc.sync.dma_start(dpat_t[:], dpat[:, :])
            nc.sync.dma_start(acol_t[:], acol[:, :])
            for L in range(1, NBLK):
                nc.sync.dma_start(rhs[L][:], y[L])

            for q, (lhs_idx, L, is_diag) in enumerate(quads):
                lhsrc = rhs[0] if lhs_idx == 0 else rhs[8]
                ps = psp.tile([P, 4 * BLK], f32, tag="ps", name=f"ps_{q}")
                for rt_ in range(4):
                    seg = ps[:, rt_ * BLK : (rt_ + 1) * BLK]
                    for k in range(KCH):
                        nc.tensor.matmul(
                            seg,
                            lhsrc[:, k * BLK + rt_ * P : k * BLK + (rt_ + 1) * P],
                            rhs[L][:, k * BLK : (k + 1) * BLK],
                            start=(k == 0),
                            stop=(k == KCH - 1) and not is_diag,
                        )
                    if is_diag:
                        nc.tensor.matmul(
                            seg,
                            ident_t[:],
                            dpat_t[:, rt_ * BLK : (rt_ + 1) * BLK],
                            start=False,
                            stop=True,
                        )
                e = ep.tile([P, 4 * BLK], bf16, tag="e", name=f"e_{q}")
                nc.scalar.activation(e[:], ps[:], Exp)
                # weight by A_j (same column block for all 4 row subtiles:
                # broadcast the 512-wide slice across the 4 banks)
                ew = ep.tile([P, 4 * BLK], bf16, tag="ew", name=f"ew_{q}")
                a_b = acol_t[:, None, L * BLK : (L + 1) * BLK].to_broadcast(
                    (P, 4, BLK)
                )
                nc.vector.tensor_tensor(
                    ew[:].rearrange("p (r b) -> p r b", r=4), e[:].rearrange("p (r b) -> p r b", r=4), a_b, mult
                )
                for rt_ in range(4):
                    t = 4 * q + rt_
                    nc.vector.reduce_sum(
                        stats_t[:, t : t + 1],
                        ew[:, rt_ * BLK : (rt_ + 1) * BLK],
                        axis=X,
                    )

            nc.sync.dma_start(stats[:, :], stats_t[:])

    nc.compile()
    return nc


def _host_inputs(z: np.ndarray):
    """Build the per-core input maps from the full z [8192, 512] fp32."""
    bf16 = ml_dtypes.bfloat16
    z64 = z.astype(np.float64)
    s = math.sqrt(2.0 / TAU)
    yT64 = (z64 * s).T  # [512, 8192]
    sqy64 = (2.0 / TAU) * np.sum(z64 * z64, axis=1)  # [8192]
    v64 = -0.5 * sqy64  # -sqy_j / 2

    ident = np.eye(P, dtype=np.float32).astype(bf16)
    dpat = np.zeros((P, 4 * BLK), dtype=np.float32)
    for rt_ in range(4):
        for p in range(P):
            dpat[p, rt_ * BLK + rt_ * P + p] = DIAG_NEG
    dpat = dpat.astype(bf16)

    in_maps = []
    amaps = []
    for c in range(NCORES):
        yr = np.roll(yT64, -BLK * c, axis=1).astype(np.float32).astype(bf16)
        # [512, 8192] -> [L=16, p=128, k=4, c=512] -> [16, 128, 2048]
        yl = np.ascontiguousarray(
            yr.reshape(KCH, P, NBLK, BLK).transpose(2, 1, 0, 3).reshape(
                NBLK, P, KCH * BLK
            )
        )

        vr = np.roll(v64, -BLK * c)
        acol = np.broadcast_to(
            np.exp(vr).astype(np.float32).astype(bf16)[None, :], (P, N)
        )

        # host-side row factor exp(-sqy_i/2) per (partition, psum tile)
        amap = np.empty((P, NT), dtype=np.float64)
        for q in range(NQ):
            lhs_idx = 0 if q < 9 else 1
            for rt_ in range(4):
                base = BLK * (c + 8 * lhs_idx) + rt_ * P
                amap[:, 4 * q + rt_] = np.exp(v64[base : base + P])
        amaps.append(amap)

        in_maps.append(
            {
                "y": yl,
                "acol": np.ascontiguousarray(acol),
                "ident": ident,
                "dpat": dpat,
            }
        )
    return in_maps, amaps


def _reduce(results, amaps) -> np.ndarray:
    w = np.ones(NT)
    w *= 2.0
    for q in DIAG_QUADS:
        w[4 * q : 4 * q + 4] = 1.0
    total = 0.0
    for out_map, amap in zip(results, amaps):
        st = out_map["stats"].astype(np.float64)  # [P, NT]
        total += ((st * amap).sum(axis=0) * w).sum()
    mean = total / (float(N) * float(N - 1))
    return np.array(math.log(mean), dtype=np.float32)


def run(z: np.ndarray, trace: bool = False, tmpdir=None):
    from concourse.bass_utils import run_bass_kernel_spmd

    if "nc" not in _cache:
        _cache["nc"] = _build_nc()
    nc = _cache["nc"]
    in_maps, amaps = _host_inputs(np.asarray(z, dtype=np.float32))
    res = run_bass_kernel_spmd(
        nc, in_maps, core_ids=list(range(NCORES)), trace=trace, tmpdir=tmpdir
    )
    return _reduce(res.results, amaps), res


def kernel(z: np.ndarray) -> np.ndarray:
    out, _ = run(z, trace=False)
    return out
